# revision 1
# baseline (speedup 1.0000x reference)
"""CirConv2d kernel for 8 Trainium2 NeuronCores.

Strategy: data-parallel over batch (2 images per core). The circulant
weight synthesis (softmax-mixed block-circulant projections, ~2.25 MB)
is computed on host in numpy (it is 1.5% of the FLOPs); the 3x3 conv —
the dominant cost — runs on device as 9-tap PSUM-accumulated matmuls
over input-channel tiles, using float32r matmuls (full-rate fp32 path
on the PE for moving dim >= 256).
"""

import sys
import numpy as np

sys.path.insert(0, "/opt/trn_rl_repo")

N_CORES = 8
B, C, H = 16, 256, 56
O, I, KS = 256, 256, 3
BPC = B // N_CORES  # batches per core
SEARCH_SPACE = [1, 2, 4, 8, 16, 32, 64]
GUMBEL_SCALE = 1e-4
TAU = 1.0

HP = H + 2            # padded width 58
NPIX = HP * HP        # 3364
ROWS_PER_CHUNK = 8
NCHUNK = H // ROWS_PER_CHUNK  # 7
NCOL = ROWS_PER_CHUNK * H     # 448 output pixels per matmul

_CACHE = {}


def _synth_weight_host(weight, alphas_after):
    w = alphas_after[0] * weight
    for idx, b in enumerate(SEARCH_SPACE[1:], start=1):
        q, p = O // b, I // b
        tmp = weight.reshape(q, b, p, b, KS, KS).transpose(0, 2, 1, 3, 4, 5)
        ii = np.arange(b)[:, None]
        jj = np.arange(b)[None, :]
        rot = tmp[:, :, ii, (ii + jj) % b]          # q,p,b,b,k,k
        cir = rot.mean(axis=2, dtype=np.float32)     # q,p,b,k,k
        out = cir[:, :, (jj - ii) % b]               # q,p,b,b,k,k
        out = out.transpose(0, 2, 1, 3, 4, 5).reshape(O, I, KS, KS)
        w = w + alphas_after[idx] * out
    return w.astype(np.float32)


def _build(reps_dyn=0):
    import concourse.bacc as bacc
    import concourse.bass as bass
    import concourse.mybir as mybir
    from concourse.tile import TileContext

    AP = bass.AP
    f32 = mybir.dt.float32
    f32r = mybir.dt.float32r

    nc = bacc.Bacc("TRN2", target_bir_lowering=False, debug=False,
                   num_devices=N_CORES)
    xin = nc.declare_dram_parameter("x", [BPC, C, H, H], f32, isOutput=False)
    win = nc.declare_dram_parameter("wsynT", [I, O * 9], f32, isOutput=False)
    yout = nc.declare_dram_parameter("y", [BPC, O, H, H], f32, isOutput=True)

    with TileContext(nc) as tc:
        with tc.tile_pool(name="persist", bufs=1) as pp, \
             tc.tile_pool(name="psum", bufs=4, space="PSUM") as psp, \
             tc.tile_pool(name="load", bufs=2) as ldp, \
             tc.tile_pool(name="stage", bufs=4) as stp:
            # small zero tile used to zero the f32r pad borders
            zt = pp.tile([128, 2 * HP], f32, tag="zt")
            nc.vector.memset(zt[:], 0.0)
            # synthesized weight, transposed: [i, o*9+tap], rounded to f32r
            wt = []
            for it in range(2):
                ws = ldp.tile([128, O * 9], f32, tag="wstage")
                nc.sync.dma_start(out=ws[:], in_=win[it * 128:(it + 1) * 128, :])
                t = pp.tile([128, O * 9], f32r, tag=f"w{it}")
                nc.vector.tensor_copy(t[:], ws[:])
                wt.append(t)
            # zero-padded input images: [b][it] -> [128, 58*58] f32r.
            # x DMA lands contiguous; the pad placement + f32r rounding happen
            # in one DVE copy; borders are zeroed from the f32 zero tile.
            xp = [[None] * 2 for _ in range(BPC)]
            for b in range(BPC):
                for it in range(2):
                    t = pp.tile([128, NPIX], f32r, tag=f"xp{b}{it}")
                    ta = t[:]
                    nc.vector.tensor_copy(
                        AP(ta.tensor, ta.offset, [[NPIX, 128], [1, HP]]),
                        zt[:, 0:HP])
                    nc.vector.tensor_copy(
                        AP(ta.tensor, ta.offset + (HP - 1) * HP,
                           [[NPIX, 128], [1, HP]]),
                        zt[:, 0:HP])
                    nc.vector.tensor_copy(
                        AP(ta.tensor, ta.offset, [[NPIX, 128], [HP, HP], [HP - 1, 2]]),
                        zt[:, 0:2 * HP])
                    xs = ldp.tile([128, H * H], f32, tag="xstage")
                    nc.sync.dma_start(out=xs[:], in_=xin[b, it * 128:(it + 1) * 128, :, :])
                    dst = AP(ta.tensor, ta.offset + HP + 1,
                             [[NPIX, 128], [HP, H], [1, H]])
                    nc.vector.tensor_copy(dst, xs[:])
                    xp[b][it] = t
            def conv_body():
                for b in range(BPC):
                    for ot in range(2):
                        for ch in range(NCHUNK):
                            ps = psp.tile([128, NCOL], f32, tag="ps")
                            idx = 0
                            for it in range(2):
                                wap = wt[it][:]
                                for kh in range(3):
                                    for kw in range(3):
                                        t = kh * 3 + kw
                                        lhsT = AP(wap.tensor,
                                                  wap.offset + ot * 128 * 9 + t,
                                                  [[O * 9, 128], [9, 128]])
                                        xap = xp[b][it][:]
                                        rhs = AP(xap.tensor,
                                                 xap.offset + (ch * ROWS_PER_CHUNK + kh) * HP + kw,
                                                 [[NPIX, 128], [HP, ROWS_PER_CHUNK], [1, H]])
                                        nc.tensor.matmul(ps[:], lhsT, rhs,
                                                         start=(idx == 0),
                                                         stop=(idx == 17))
                                        idx += 1
                            st = stp.tile([128, NCOL], f32, tag="st")
                            nc.scalar.copy(out=st[:], in_=ps[:])
                            ybase = (b * O + ot * 128) * (H * H) + ch * NCOL
                            dst = AP(yout[:].tensor, ybase, [[H * H, 128], [1, NCOL]])
                            nc.sync.dma_start(out=dst, in_=st[:])

            if reps_dyn:
                with tc.For_i(0, reps_dyn, 1):
                    conv_body()
            else:
                conv_body()
    nc.compile()
    return nc


def _get_nc():
    if "nc" not in _CACHE:
        _CACHE["nc"] = _build()
    return _CACHE["nc"]


def _host_prep(x, weight, alphas, gumbels):
    x = np.ascontiguousarray(np.asarray(x, dtype=np.float32))
    weight = np.asarray(weight, dtype=np.float32)
    alphas = np.asarray(alphas, dtype=np.float32)
    gumbels = np.asarray(gumbels, dtype=np.float32)

    a = (alphas + np.float32(GUMBEL_SCALE) * gumbels) / np.float32(TAU)
    a = a - a.max()
    e = np.exp(a, dtype=np.float32)
    alphas_after = (e / e.sum(dtype=np.float32)).astype(np.float32)

    w = _synth_weight_host(weight, alphas_after)  # [O, I, 3, 3]
    wsynT = np.ascontiguousarray(
        w.reshape(O, I, 9).transpose(1, 0, 2).reshape(I, O * 9).astype(np.float32))
    return x, wsynT


def kernel(x, weight, alphas, gumbels):
    x, wsynT = _host_prep(x, weight, alphas, gumbels)
    nc = _get_nc()

    from concourse.bass_utils import run_bass_kernel_spmd
    in_maps = [{"x": x[i * BPC:(i + 1) * BPC], "wsynT": wsynT}
               for i in range(N_CORES)]
    res = run_bass_kernel_spmd(nc, in_maps, list(range(N_CORES)))
    out = np.concatenate([res.results[i]["y"] for i in range(N_CORES)], axis=0)
    return np.ascontiguousarray(out.astype(np.float32))



# revision 5
# speedup vs baseline: 1.1033x; 1.1033x over previous
"""CirConv2d kernel for 8 Trainium2 NeuronCores.

Strategy: data-parallel over batch (2 images per core). The circulant
weight synthesis (softmax-mixed block-circulant projections, ~2.25 MB)
is computed on host in numpy (it is 1.5% of the FLOPs); the 3x3 conv —
the dominant cost — runs on device as 9-tap PSUM-accumulated matmuls
over input-channel tiles in bf16. bf16 weights get separate LDWEIGHTS
instructions that the PE's reorder window overlaps with in-flight
matmuls (f32r matmuls must self-load weights serially), so the matmul
stream runs at ~N cycles per matmul instead of N+128.
"""

import sys
import numpy as np
import ml_dtypes

sys.path.insert(0, "/opt/trn_rl_repo")

N_CORES = 8
B, C, H = 16, 256, 56
O, I, KS = 256, 256, 3
BPC = B // N_CORES  # batches per core
SEARCH_SPACE = [1, 2, 4, 8, 16, 32, 64]
GUMBEL_SCALE = 1e-4
TAU = 1.0

HP = H + 2            # padded width 58
NPIX = HP * HP        # 3364
ROWS_PER_CHUNK = 8
NCHUNK = H // ROWS_PER_CHUNK  # 7
NCOL = ROWS_PER_CHUNK * H     # 448 output pixels per matmul

BF16 = ml_dtypes.bfloat16

# device-side tensor declarations, exported for test.py's dyn harness
DEV_INPUTS = {
    "x": ((BPC, C, H, H), BF16),
    "wsynT": ((I, 9 * O), BF16),
}
DEV_OUTPUT = ("y", (BPC, O, H, H), BF16)

_CACHE = {}


def _synth_weight_host(weight, alphas_after):
    w = alphas_after[0] * weight
    for idx, b in enumerate(SEARCH_SPACE[1:], start=1):
        q, p = O // b, I // b
        tmp = weight.reshape(q, b, p, b, KS, KS).transpose(0, 2, 1, 3, 4, 5)
        ii = np.arange(b)[:, None]
        jj = np.arange(b)[None, :]
        rot = tmp[:, :, ii, (ii + jj) % b]          # q,p,b,b,k,k
        cir = rot.mean(axis=2, dtype=np.float32)     # q,p,b,k,k
        out = cir[:, :, (jj - ii) % b]               # q,p,b,b,k,k
        out = out.transpose(0, 2, 1, 3, 4, 5).reshape(O, I, KS, KS)
        w = w + alphas_after[idx] * out
    return w.astype(np.float32)


def _build(reps_dyn=0):
    import concourse.bacc as bacc
    import concourse.bass as bass
    import concourse.mybir as mybir
    from concourse.tile import TileContext

    AP = bass.AP
    bf16 = mybir.dt.bfloat16
    f32 = mybir.dt.float32

    nc = bacc.Bacc("TRN2", target_bir_lowering=False, debug=False,
                   num_devices=N_CORES)
    xin = nc.declare_dram_parameter("x", list(DEV_INPUTS["x"][0]), bf16,
                                    isOutput=False)
    win = nc.declare_dram_parameter("wsynT", list(DEV_INPUTS["wsynT"][0]),
                                    bf16, isOutput=False)
    yout = nc.declare_dram_parameter("y", list(DEV_OUTPUT[1]), bf16,
                                     isOutput=True)

    with TileContext(nc) as tc:
        with tc.tile_pool(name="persist", bufs=1) as pp, \
             tc.tile_pool(name="psum", bufs=2, space="PSUM") as psp, \
             tc.tile_pool(name="stage", bufs=4) as stp:
            # small zero tile used to zero the pad borders
            zt = pp.tile([128, 2 * HP], bf16, tag="zt")
            nc.vector.memset(zt[:], 0.0)
            # synthesized weight, transposed: [i, (tap*O + o)] bf16 so each
            # (it, ot, tap) weight slice is 128 contiguous elements (FWL).
            wt = []
            for it in range(2):
                t = pp.tile([128, 9 * O], bf16, tag=f"w{it}")
                nc.sync.dma_start(out=t[:], in_=win[it * 128:(it + 1) * 128, :])
                wt.append(t)
            # zero-padded input images: [b][it] -> [128, 58*58] bf16.
            xp = [[None] * 2 for _ in range(BPC)]
            for b in range(BPC):
                for it in range(2):
                    t = pp.tile([128, NPIX], bf16, tag=f"xp{b}{it}")
                    ta = t[:]
                    nc.vector.tensor_copy(
                        AP(ta.tensor, ta.offset, [[NPIX, 128], [1, HP]]),
                        zt[:, 0:HP])
                    nc.vector.tensor_copy(
                        AP(ta.tensor, ta.offset + (HP - 1) * HP,
                           [[NPIX, 128], [1, HP]]),
                        zt[:, 0:HP])
                    nc.vector.tensor_copy(
                        AP(ta.tensor, ta.offset, [[NPIX, 128], [HP, HP], [HP - 1, 2]]),
                        zt[:, 0:2 * HP])
                    xs = stp.tile([128, H * H], bf16, tag="xstage")
                    nc.sync.dma_start(out=xs[:], in_=xin[b, it * 128:(it + 1) * 128, :, :])
                    dst = AP(ta.tensor, ta.offset + HP + 1,
                             [[NPIX, 128], [HP, H], [1, H]])
                    nc.vector.tensor_copy(dst, xs[:])
                    xp[b][it] = t

            def conv_body():
                # Blocks of up to 4 PSUM accumulation groups run in parallel so
                # each loaded weight tile serves 4 consecutive matmuls (walrus
                # skips re-issuing LDWEIGHTS when the weights AP is unchanged).
                for ot in range(2):
                    for blk in range(4):
                        chs = [2 * blk, 2 * blk + 1] if blk < 3 else [6]
                        groups = [(b, ch) for b in range(BPC) for ch in chs]
                        ps = {}
                        for gi, g in enumerate(groups):
                            ps[g] = psp.tile([128, NCOL], f32, tag=f"ps{gi}",
                                             name=f"ps{gi}")
                        for it in range(2):
                            wap = wt[it][:]
                            for kh in range(3):
                                for kw in range(3):
                                    tap = kh * 3 + kw
                                    idx = it * 9 + tap
                                    lhsT = AP(wap.tensor,
                                              wap.offset + tap * O + ot * 128,
                                              [[9 * O, 128], [1, 128]])
                                    for (b, ch) in groups:
                                        xap = xp[b][it][:]
                                        rhs = AP(xap.tensor,
                                                 xap.offset + (ch * ROWS_PER_CHUNK + kh) * HP + kw,
                                                 [[NPIX, 128], [HP, ROWS_PER_CHUNK], [1, H]])
                                        nc.tensor.matmul(ps[(b, ch)][:], lhsT, rhs,
                                                         start=(idx == 0),
                                                         stop=(idx == 17))
                        for (b, ch) in groups:
                            st = stp.tile([128, NCOL], bf16, tag="st")
                            nc.scalar.copy(out=st[:], in_=ps[(b, ch)][:])
                            ybase = (b * O + ot * 128) * (H * H) + ch * NCOL
                            dst = AP(yout[:].tensor, ybase, [[H * H, 128], [1, NCOL]])
                            nc.sync.dma_start(out=dst, in_=st[:])

            if reps_dyn:
                with tc.For_i(0, reps_dyn, 1):
                    conv_body()
            else:
                conv_body()
    nc.compile()
    return nc


def _get_nc():
    if "nc" not in _CACHE:
        _CACHE["nc"] = _build()
    return _CACHE["nc"]


def _host_prep(x, weight, alphas, gumbels):
    x = np.asarray(x, dtype=np.float32)
    weight = np.asarray(weight, dtype=np.float32)
    alphas = np.asarray(alphas, dtype=np.float32)
    gumbels = np.asarray(gumbels, dtype=np.float32)

    a = (alphas + np.float32(GUMBEL_SCALE) * gumbels) / np.float32(TAU)
    a = a - a.max()
    e = np.exp(a, dtype=np.float32)
    alphas_after = (e / e.sum(dtype=np.float32)).astype(np.float32)

    w = _synth_weight_host(weight, alphas_after)  # [O, I, 3, 3]
    # [I, tap*O + o] layout: contiguous O for each (i, tap)
    wsynT = np.ascontiguousarray(
        w.reshape(O, I, 9).transpose(1, 2, 0).reshape(I, 9 * O))
    return (np.ascontiguousarray(x).astype(BF16),
            wsynT.astype(BF16))


def kernel(x, weight, alphas, gumbels):
    x, wsynT = _host_prep(x, weight, alphas, gumbels)
    nc = _get_nc()

    from concourse.bass_utils import run_bass_kernel_spmd
    in_maps = [{"x": x[i * BPC:(i + 1) * BPC], "wsynT": wsynT}
               for i in range(N_CORES)]
    res = run_bass_kernel_spmd(nc, in_maps, list(range(N_CORES)))
    out = np.concatenate([res.results[i]["y"] for i in range(N_CORES)], axis=0)
    return np.ascontiguousarray(out.astype(np.float32))


# revision 9
# speedup vs baseline: 1.3501x; 1.2237x over previous
"""CirConv2d kernel for 8 Trainium2 NeuronCores.

Strategy: data-parallel over batch (2 images per core). The circulant
weight synthesis (softmax-mixed block-circulant projections) happens on
host (1.5% of FLOPs). The 3x3 conv runs as 1-D Winograd F(2,3) along W
(4 frequency taps replace the 3 kw taps while producing 2 output
columns at once = 1.5x fewer PE MACs) with direct 3-tap accumulation
along H, all in bf16 with f32 PSUM accumulation:

  V[c,r,t,f]  = fwd transform of padded x rows (DVE adds, bf16)
  Y[o,h,t,f] += sum_{c,kh} U[f,kh,o,c] V[c,h+kh,t,f]   (PE matmuls)
  y[o,h,2t]   = Y0+Y1+Y2 ; y[o,h,2t+1] = Y1-Y2-Y3      (DVE, PSUM->SBUF)
"""

import sys
import numpy as np
import ml_dtypes

sys.path.insert(0, "/opt/trn_rl_repo")

N_CORES = 8
B, C, H = 16, 256, 56
O, I, KS = 256, 256, 3
BPC = B // N_CORES  # batches per core
SEARCH_SPACE = [1, 2, 4, 8, 16, 32, 64]
GUMBEL_SCALE = 1e-4
TAU = 1.0

HP = H + 2            # padded width/height 58
NPIX = HP * HP        # 3364
TW = H // 2           # 28 winograd tiles along W
NF = 4                # F(2,3) frequencies
ROWS_PER_CHUNK = 8
NCHUNK = H // ROWS_PER_CHUNK  # 7
NCOL = ROWS_PER_CHUNK * TW * BPC   # 448 domain cols per matmul (8h x 28t x 2img)
VROW = TW              # 28 values per padded row per (f, img)
VBLK = HP * VROW       # 58*28 = 1624 per (f, img)
VIMG = 2 * VBLK        # per f: both images

BF16 = ml_dtypes.bfloat16

DEV_INPUTS = {
    "x": ((BPC, C, H, H), BF16),
    "wsynT": ((I, NF * 3 * O), BF16),   # [i, ((f*3+kh)*O + o)]
}
DEV_OUTPUT = ("y", (BPC, O, H, H), BF16)

_CACHE = {}


def _synth_weight_host(weight, alphas_after):
    w = alphas_after[0] * weight
    for idx, b in enumerate(SEARCH_SPACE[1:], start=1):
        q, p = O // b, I // b
        tmp = weight.reshape(q, b, p, b, KS, KS).transpose(0, 2, 1, 3, 4, 5)
        ii = np.arange(b)[:, None]
        jj = np.arange(b)[None, :]
        rot = tmp[:, :, ii, (ii + jj) % b]          # q,p,b,b,k,k
        cir = rot.mean(axis=2, dtype=np.float32)     # q,p,b,k,k
        out = cir[:, :, (jj - ii) % b]               # q,p,b,b,k,k
        out = out.transpose(0, 2, 1, 3, 4, 5).reshape(O, I, KS, KS)
        w = w + alphas_after[idx] * out
    return w.astype(np.float32)


def _build(reps_dyn=0):
    import concourse.bacc as bacc
    import concourse.bass as bass
    import concourse.mybir as mybir
    from concourse.tile import TileContext

    AP = bass.AP
    bf16 = mybir.dt.bfloat16
    f32 = mybir.dt.float32

    nc = bacc.Bacc("TRN2", target_bir_lowering=False, debug=False,
                   num_devices=N_CORES)
    xin = nc.declare_dram_parameter("x", list(DEV_INPUTS["x"][0]), bf16,
                                    isOutput=False)
    win = nc.declare_dram_parameter("wsynT", list(DEV_INPUTS["wsynT"][0]),
                                    bf16, isOutput=False)
    yout = nc.declare_dram_parameter("y", list(DEV_OUTPUT[1]), bf16,
                                     isOutput=True)

    with TileContext(nc) as tc:
        with tc.tile_pool(name="persist", bufs=1) as pp, \
             tc.tile_pool(name="psum", bufs=2, space="PSUM") as psp, \
             tc.tile_pool(name="stage", bufs=4) as stp:
            zt = pp.tile([128, 2 * HP], bf16, tag="zt")
            nc.vector.memset(zt[:], 0.0)
            # winograd weights: [i, (f*3+kh)*O + o], contiguous 128-wide
            wt = []
            for it in range(2):
                t = pp.tile([128, NF * 3 * O], bf16, tag=f"w{it}", name="wtile")
                nc.sync.dma_start(out=t[:], in_=win[it * 128:(it + 1) * 128, :])
                wt.append(t)
            # zero-padded input images, both images in one tile per it:
            # xp[it][128, 2*NPIX]
            xp = []
            for it in range(2):
                t = pp.tile([128, 2 * NPIX], bf16, tag=f"xp{it}", name="xptile")
                for b in range(BPC):
                    ta = t[:]
                    base = b * NPIX
                    nc.vector.tensor_copy(
                        AP(ta.tensor, ta.offset + base, [[2 * NPIX, 128], [1, HP]]),
                        zt[:, 0:HP])
                    nc.vector.tensor_copy(
                        AP(ta.tensor, ta.offset + base + (HP - 1) * HP,
                           [[2 * NPIX, 128], [1, HP]]),
                        zt[:, 0:HP])
                    nc.vector.tensor_copy(
                        AP(ta.tensor, ta.offset + base,
                           [[2 * NPIX, 128], [HP, HP], [HP - 1, 2]]),
                        zt[:, 0:2 * HP])
                    xs = stp.tile([128, H * H], bf16, tag="xstage", name="xstage")
                    nc.sync.dma_start(out=xs[:], in_=xin[b, it * 128:(it + 1) * 128, :, :])
                    dst = AP(ta.tensor, ta.offset + base + HP + 1,
                             [[2 * NPIX, 128], [HP, H], [1, H]])
                    nc.vector.tensor_copy(dst, xs[:])
                xp.append(t)
            def fwd_transform(vt):
                # V[...,f] from x cols 2t+k: f0=d0-d2 f1=d1+d2 f2=d2-d1 f3=d1-d3
                for it in range(2):
                    va = vt[it][:]
                    xa = xp[it][:]

                    def vout(f):
                        return AP(va.tensor, va.offset + f * VIMG,
                                  [[NF * VIMG, 128], [VBLK, 2], [VROW, HP], [1, TW]])

                    def xin_(k):
                        return AP(xa.tensor, xa.offset + k,
                                  [[2 * NPIX, 128], [NPIX, 2], [HP, HP], [2, TW]])

                    nc.vector.tensor_sub(vout(0), xin_(0), xin_(2))
                    nc.vector.tensor_add(vout(1), xin_(1), xin_(2))
                    nc.vector.tensor_sub(vout(2), xin_(2), xin_(1))
                    nc.vector.tensor_sub(vout(3), xin_(1), xin_(3))

            def conv_body():
                # V tiles double-buffered so next rep's transform overlaps
                # this rep's matmuls: V[it][128, f(4) x img(2) x r(58) x t(28)]
                vt = [stp.tile([128, NF * VIMG], bf16, tag=f"v{it}",
                               name=f"v{it}", bufs=2) for it in range(2)]
                fwd_transform(vt)
                for ot in range(2):
                    for ch in range(NCHUNK):
                        ps = [psp.tile([128, NCOL], f32, tag=f"ps{f}",
                                       name=f"ps{f}") for f in range(NF)]
                        for f in range(NF):
                            idx = 0
                            for it in range(2):
                                wap = wt[it][:]
                                for kh in range(3):
                                    lhsT = AP(wap.tensor,
                                              wap.offset + (f * 3 + kh) * O + ot * 128,
                                              [[NF * 3 * O, 128], [1, 128]])
                                    va = vt[it][:]
                                    rhs = AP(va.tensor,
                                             va.offset + f * VIMG + (ch * ROWS_PER_CHUNK + kh) * VROW,
                                             [[NF * VIMG, 128], [VBLK, 2],
                                              [VROW, ROWS_PER_CHUNK], [1, TW]])
                                    nc.tensor.matmul(ps[f][:], lhsT, rhs,
                                                     start=(idx == 0),
                                                     stop=(idx == 5))
                                    idx += 1
                        # inverse transform: even = (Y0+Y1)+Y2, odd = (Y1-Y2)-Y3
                        # DVE may read at most one PSUM operand per op, so Y0
                        # and Y2 are first staged to SBUF by the scalar engine.
                        c0 = stp.tile([128, NCOL], f32, tag="c0", name="c0")
                        c2 = stp.tile([128, NCOL], f32, tag="c2", name="c2")
                        nc.scalar.copy(out=c0[:], in_=ps[0][:])
                        nc.scalar.copy(out=c2[:], in_=ps[2][:])
                        tmp = stp.tile([128, 2 * NCOL], f32, tag="tmp", name="tmp")
                        st = stp.tile([128, ROWS_PER_CHUNK * H * BPC], bf16,
                                      tag="st", name="st")
                        ta = tmp[:]
                        te = AP(ta.tensor, ta.offset, [[2 * NCOL, 128], [1, NCOL]])
                        to = AP(ta.tensor, ta.offset + NCOL, [[2 * NCOL, 128], [1, NCOL]])
                        nc.vector.tensor_add(te, c0[:], ps[1][:])
                        nc.vector.tensor_sub(to, ps[1][:], c2[:])
                        sa = st[:]
                        # st layout: [img(2), h(8), w(56)] per partition
                        se = AP(sa.tensor, sa.offset, [[2 * NCOL, 128],
                                                       [ROWS_PER_CHUNK * H, 2],
                                                       [H, ROWS_PER_CHUNK], [2, TW]])
                        so = AP(sa.tensor, sa.offset + 1, [[2 * NCOL, 128],
                                                           [ROWS_PER_CHUNK * H, 2],
                                                           [H, ROWS_PER_CHUNK], [2, TW]])
                        nc.vector.tensor_add(se, te, c2[:])
                        nc.vector.tensor_sub(so, to, ps[3][:])
                        for b in range(BPC):
                            ybase = (b * O + ot * 128) * (H * H) + ch * ROWS_PER_CHUNK * H
                            dst = AP(yout[:].tensor, ybase,
                                     [[H * H, 128], [1, ROWS_PER_CHUNK * H]])
                            nc.sync.dma_start(
                                out=dst,
                                in_=AP(sa.tensor, sa.offset + b * ROWS_PER_CHUNK * H,
                                       [[2 * NCOL, 128], [1, ROWS_PER_CHUNK * H]]))

            if reps_dyn:
                with tc.For_i(0, reps_dyn, 1):
                    conv_body()
            else:
                conv_body()
    nc.compile()
    return nc


def _get_nc():
    if "nc" not in _CACHE:
        _CACHE["nc"] = _build()
    return _CACHE["nc"]


def _host_prep(x, weight, alphas, gumbels):
    x = np.asarray(x, dtype=np.float32)
    weight = np.asarray(weight, dtype=np.float32)
    alphas = np.asarray(alphas, dtype=np.float32)
    gumbels = np.asarray(gumbels, dtype=np.float32)

    a = (alphas + np.float32(GUMBEL_SCALE) * gumbels) / np.float32(TAU)
    a = a - a.max()
    e = np.exp(a, dtype=np.float32)
    alphas_after = (e / e.sum(dtype=np.float32)).astype(np.float32)

    w = _synth_weight_host(weight, alphas_after)  # [O, I, 3, 3]
    # winograd weight transform along kw: U[f,kh,i,o], layout [i, (f*3+kh)*O+o]
    G = np.array([[1, 0, 0], [0.5, 0.5, 0.5], [0.5, -0.5, 0.5], [0, 0, 1]],
                 np.float32)
    U = np.einsum('fk,oihk->ifho', G, w)          # i, f, kh, o
    wsynT = np.ascontiguousarray(U.reshape(I, NF * 3 * O))
    return (np.ascontiguousarray(x).astype(BF16),
            wsynT.astype(BF16))


def kernel(x, weight, alphas, gumbels):
    x, wsynT = _host_prep(x, weight, alphas, gumbels)
    nc = _get_nc()

    from concourse.bass_utils import run_bass_kernel_spmd
    in_maps = [{"x": x[i * BPC:(i + 1) * BPC], "wsynT": wsynT}
               for i in range(N_CORES)]
    res = run_bass_kernel_spmd(nc, in_maps, list(range(N_CORES)))
    out = np.concatenate([res.results[i]["y"] for i in range(N_CORES)], axis=0)
    return np.ascontiguousarray(out.astype(np.float32))


# revision 15
# speedup vs baseline: 1.5456x; 1.1448x over previous
"""CirConv2d kernel for 8 Trainium2 NeuronCores.

Strategy: data-parallel over batch (2 images per core). The circulant
weight synthesis (softmax-mixed block-circulant projections) happens on
host (1.5% of FLOPs). The 3x3 conv runs as 1-D Winograd F(2,3) along W
(4 frequency taps replace the 3 kw taps while producing 2 output
columns at once = 1.5x fewer PE MACs) with direct 3-tap accumulation
along H, all in bf16 with f32 PSUM accumulation:

  V[c,r,t,f]  = fwd transform of padded x rows (DVE adds, bf16)
  Y[o,h,t,f] += sum_{c,kh} U[f,kh,o,c] V[c,h+kh,t,f]   (PE matmuls)
  y[o,h,2t]   = Y0+Y1+Y2 ; y[o,h,2t+1] = Y1-Y2-Y3      (DVE, PSUM->SBUF)
"""

import sys
import numpy as np
import ml_dtypes

sys.path.insert(0, "/opt/trn_rl_repo")

N_CORES = 8
B, C, H = 16, 256, 56
O, I, KS = 256, 256, 3
BPC = B // N_CORES  # batches per core
SEARCH_SPACE = [1, 2, 4, 8, 16, 32, 64]
GUMBEL_SCALE = 1e-4
TAU = 1.0

HP = H + 2            # padded width/height 58
NPIX = HP * HP        # 3364
TW = H // 2           # 28 winograd tiles along W
NF = 4                # F(2,3) frequencies
ROWS_PER_CHUNK = 8
NCHUNK = H // ROWS_PER_CHUNK  # 7
NCOL = ROWS_PER_CHUNK * TW * BPC   # 448 domain cols per matmul (8h x 28t x 2img)
VROW = TW              # 28 values per padded row per (f, img)
VBLK = HP * VROW       # 58*28 = 1624 per (f, img)
VIMG = 2 * VBLK        # per f: both images

BF16 = ml_dtypes.bfloat16

DEV_INPUTS = {
    "x": ((BPC, C, H, H), BF16),
    "wsynT": ((I, NF * 3 * O), BF16),   # [i, ((f*3+kh)*O + o)]
}
DEV_OUTPUT = ("y", (BPC, O, H, H), BF16)

_CACHE = {}

# When True, rewrite the walrus invocation to pass --enable-ldw-opt=true
# (concourse hardcodes false). LDW-opt lets codegen elide/pipeline redundant
# weight loads.
ENABLE_LDW_OPT = False


def _maybe_patch_ldw_opt():
    if not ENABLE_LDW_OPT:
        return
    import concourse.bass_utils as _bu
    if getattr(_bu, "_ldw_patched", False):
        return
    _orig = _bu.run_command

    def _patched(cmd, *a, **kw):
        cmd = ["--enable-ldw-opt=true" if c == "--enable-ldw-opt=false" else c
               for c in cmd]
        return _orig(cmd, *a, **kw)

    _bu.run_command = _patched
    _bu._ldw_patched = True


def _synth_weight_host(weight, alphas_after):
    w = alphas_after[0] * weight
    for idx, b in enumerate(SEARCH_SPACE[1:], start=1):
        q, p = O // b, I // b
        tmp = weight.reshape(q, b, p, b, KS, KS).transpose(0, 2, 1, 3, 4, 5)
        ii = np.arange(b)[:, None]
        jj = np.arange(b)[None, :]
        rot = tmp[:, :, ii, (ii + jj) % b]          # q,p,b,b,k,k
        cir = rot.mean(axis=2, dtype=np.float32)     # q,p,b,k,k
        out = cir[:, :, (jj - ii) % b]               # q,p,b,b,k,k
        out = out.transpose(0, 2, 1, 3, 4, 5).reshape(O, I, KS, KS)
        w = w + alphas_after[idx] * out
    return w.astype(np.float32)


def _build(reps_dyn=0):
    _maybe_patch_ldw_opt()
    import concourse.bacc as bacc
    import concourse.bass as bass
    import concourse.mybir as mybir
    from concourse.tile import TileContext

    AP = bass.AP
    bf16 = mybir.dt.bfloat16
    f32 = mybir.dt.float32

    nc = bacc.Bacc("TRN2", target_bir_lowering=False, debug=False,
                   num_devices=N_CORES)
    xin = nc.declare_dram_parameter("x", list(DEV_INPUTS["x"][0]), bf16,
                                    isOutput=False)
    win = nc.declare_dram_parameter("wsynT", list(DEV_INPUTS["wsynT"][0]),
                                    bf16, isOutput=False)
    yout = nc.declare_dram_parameter("y", list(DEV_OUTPUT[1]), bf16,
                                     isOutput=True)

    with TileContext(nc) as tc:
        with tc.tile_pool(name="persist", bufs=1) as pp, \
             tc.tile_pool(name="psum", bufs=2, space="PSUM") as psp, \
             tc.tile_pool(name="stage", bufs=4) as stp:
            # winograd weights: [i, (f*3+kh)*O + o], contiguous 128-wide
            wt = []
            for it in range(2):
                t = pp.tile([128, NF * 3 * O], bf16, tag=f"w{it}", name="wtile")
                nc.sync.dma_start(out=t[:], in_=win[it * 128:(it + 1) * 128, :])
                wt.append(t)
            # Padded input stored as even/odd column planes so the forward
            # winograd transform reads are contiguous (keeps DVE 16-bit 2x
            # mode): xp[it][128, img(2) x plane(2) x row(58) x 29], where
            # E[e] = padded col 2e, Od[o] = padded col 2o+1.
            EP = HP // 2  # 29
            PLANE = HP * EP       # 1682
            IMGBLK = 2 * PLANE    # 3364 per image
            xp = []
            for it in range(2):
                t = pp.tile([128, 2 * IMGBLK], bf16, tag=f"xp{it}", name="xptile")
                nc.vector.memset(t[:], 0.0)
                for b in range(BPC):
                    ta = t[:]
                    base = b * IMGBLK
                    xs = stp.tile([128, H * H], bf16, tag="xstage", name="xstage", bufs=2)
                    nc.sync.dma_start(out=xs[:], in_=xin[b, it * 128:(it + 1) * 128, :, :])
                    xsa = xs[:]
                    # E plane e=1..28 <- x cols 1,3,..,55 ; rows 1..56
                    nc.vector.tensor_copy(
                        AP(ta.tensor, ta.offset + base + HP * EP * 0 + EP + 1,
                           [[2 * IMGBLK, 128], [EP, H], [1, TW]]),
                        AP(xsa.tensor, xsa.offset + 1,
                           [[H * H, 128], [H, H], [2, TW]]))
                    # Od plane o=0..27 <- x cols 0,2,..,54 ; rows 1..56
                    nc.vector.tensor_copy(
                        AP(ta.tensor, ta.offset + base + PLANE + EP,
                           [[2 * IMGBLK, 128], [EP, H], [1, TW]]),
                        AP(xsa.tensor, xsa.offset,
                           [[H * H, 128], [H, H], [2, TW]]))
                xp.append(t)

            def fwd_transform(vt):
                # V[...,f]: f0=E[t]-E[t+1] f1=Od[t]+E[t+1] f2=E[t+1]-Od[t]
                # f3=Od[t]-Od[t+1]; all reads/writes contiguous 28-wide.
                for it in range(2):
                    va = vt[it][:]
                    xa = xp[it][:]

                    def vout(f):
                        return AP(va.tensor, va.offset + f * VIMG,
                                  [[NF * VIMG, 128], [VBLK, 2], [VROW, HP], [1, TW]])

                    def pin(plane, k):
                        return AP(xa.tensor, xa.offset + plane * PLANE + k,
                                  [[2 * IMGBLK, 128], [IMGBLK, 2], [EP, HP], [1, TW]])

                    nc.vector.tensor_sub(vout(0), pin(0, 0), pin(0, 1))
                    nc.vector.tensor_add(vout(1), pin(1, 0), pin(0, 1))
                    nc.vector.tensor_sub(vout(2), pin(0, 1), pin(1, 0))
                    nc.vector.tensor_sub(vout(3), pin(1, 0), pin(1, 1))

            def conv_body():
                # V tiles double-buffered so next rep's transform overlaps
                # this rep's matmuls: V[it][128, f(4) x img(2) x r(58) x t(28)]
                vt = [stp.tile([128, NF * VIMG], bf16, tag=f"v{it}",
                               name=f"v{it}", bufs=2) for it in range(2)]
                fwd_transform(vt)
                for ot in range(2):
                    for ch in range(NCHUNK):
                        ps = [psp.tile([128, NCOL], f32, tag=f"ps{f}",
                                       name=f"ps{f}") for f in range(NF)]
                        for f in range(NF):
                            idx = 0
                            for it in range(2):
                                wap = wt[it][:]
                                for kh in range(3):
                                    lhsT = AP(wap.tensor,
                                              wap.offset + (f * 3 + kh) * O + ot * 128,
                                              [[NF * 3 * O, 128], [1, 128]])
                                    va = vt[it][:]
                                    rhs = AP(va.tensor,
                                             va.offset + f * VIMG + (ch * ROWS_PER_CHUNK + kh) * VROW,
                                             [[NF * VIMG, 128], [VBLK, 2],
                                              [VROW, ROWS_PER_CHUNK], [1, TW]])
                                    nc.tensor.matmul(ps[f][:], lhsT, rhs,
                                                     start=(idx == 0),
                                                     stop=(idx == 5))
                                    idx += 1
                        # Evacuate all four PSUM banks immediately with fast
                        # scalar-engine copies (f32 -> bf16) so the banks
                        # recycle without waiting on the inverse math; the
                        # inverse (even = (Y0+Y1)+Y2, odd = (Y1-Y2)-Y3) then
                        # runs SBUF-only in bf16, split across DVE and GpSimd.
                        cc = []
                        for f in range(NF):
                            c = stp.tile([128, NCOL], bf16, tag=f"c{f}",
                                         name=f"c{f}", bufs=2)
                            nc.scalar.copy(out=c[:], in_=ps[f][:])
                            cc.append(c)
                        tmp = stp.tile([128, 2 * NCOL], bf16, tag="tmp",
                                       name="tmp", bufs=2)
                        st = stp.tile([128, ROWS_PER_CHUNK * H * BPC], bf16,
                                      tag="st", name="st")
                        ta = tmp[:]
                        te = AP(ta.tensor, ta.offset, [[2 * NCOL, 128], [1, NCOL]])
                        to = AP(ta.tensor, ta.offset + NCOL, [[2 * NCOL, 128], [1, NCOL]])
                        nc.vector.tensor_add(te, cc[0][:], cc[1][:])
                        nc.gpsimd.tensor_sub(to, cc[1][:], cc[2][:])
                        sa = st[:]
                        # st layout: [img(2), h(8), w(56)] per partition
                        se = AP(sa.tensor, sa.offset, [[2 * NCOL, 128],
                                                       [ROWS_PER_CHUNK * H, 2],
                                                       [H, ROWS_PER_CHUNK], [2, TW]])
                        so = AP(sa.tensor, sa.offset + 1, [[2 * NCOL, 128],
                                                           [ROWS_PER_CHUNK * H, 2],
                                                           [H, ROWS_PER_CHUNK], [2, TW]])
                        nc.vector.tensor_add(se, te, cc[2][:])
                        nc.gpsimd.tensor_sub(so, to, cc[3][:])
                        for b in range(BPC):
                            ybase = (b * O + ot * 128) * (H * H) + ch * ROWS_PER_CHUNK * H
                            dst = AP(yout[:].tensor, ybase,
                                     [[H * H, 128], [1, ROWS_PER_CHUNK * H]])
                            nc.sync.dma_start(
                                out=dst,
                                in_=AP(sa.tensor, sa.offset + b * ROWS_PER_CHUNK * H,
                                       [[2 * NCOL, 128], [1, ROWS_PER_CHUNK * H]]))

            if reps_dyn:
                with tc.For_i(0, reps_dyn, 1):
                    conv_body()
            else:
                conv_body()
    nc.compile()
    return nc


def _get_nc():
    if "nc" not in _CACHE:
        _CACHE["nc"] = _build()
    return _CACHE["nc"]


def _host_prep(x, weight, alphas, gumbels):
    x = np.asarray(x, dtype=np.float32)
    weight = np.asarray(weight, dtype=np.float32)
    alphas = np.asarray(alphas, dtype=np.float32)
    gumbels = np.asarray(gumbels, dtype=np.float32)

    a = (alphas + np.float32(GUMBEL_SCALE) * gumbels) / np.float32(TAU)
    a = a - a.max()
    e = np.exp(a, dtype=np.float32)
    alphas_after = (e / e.sum(dtype=np.float32)).astype(np.float32)

    w = _synth_weight_host(weight, alphas_after)  # [O, I, 3, 3]
    # winograd weight transform along kw: U[f,kh,i,o], layout [i, (f*3+kh)*O+o]
    G = np.array([[1, 0, 0], [0.5, 0.5, 0.5], [0.5, -0.5, 0.5], [0, 0, 1]],
                 np.float32)
    U = np.einsum('fk,oihk->ifho', G, w)          # i, f, kh, o
    wsynT = np.ascontiguousarray(U.reshape(I, NF * 3 * O))
    return (np.ascontiguousarray(x).astype(BF16),
            wsynT.astype(BF16))


def kernel(x, weight, alphas, gumbels):
    x, wsynT = _host_prep(x, weight, alphas, gumbels)
    nc = _get_nc()

    from concourse.bass_utils import run_bass_kernel_spmd
    in_maps = [{"x": x[i * BPC:(i + 1) * BPC], "wsynT": wsynT}
               for i in range(N_CORES)]
    res = run_bass_kernel_spmd(nc, in_maps, list(range(N_CORES)))
    out = np.concatenate([res.results[i]["y"] for i in range(N_CORES)], axis=0)
    return np.ascontiguousarray(out.astype(np.float32))
